# revision 55
# baseline (speedup 1.0000x reference)
"""Causal GQA attention (prefill) on 8 TRN2 NeuronCores.

Problem: B=2, S=2048, H=32 query heads, Hk=8 kv heads, D=128, f32 I/O.
Sharding: tensor-parallel over heads -- core c gets query heads [4c, 4c+4)
and kv head c. Attention is fully independent per head: no collectives.

Per-core kernel: 8 instances of causal attention, one per (batch, qhead),
processed as a software-pipelined stream of 32 (instance, superblock)
items. Engine budget drives the design (PE matmul streaming ~117us is
the floor; ScalarE exp and the DVE share the elementwise work; the PE
is strictly in-order so the emission order IS the schedule):
  - Q and K are pre-cast to bf16 AND pre-transposed to [d, s] layout on
    the host, so Q^T/K^T stream straight from DRAM into SBUF with plain
    contiguous DMAs: zero PE transposes, zero DVE copies, half the DMA
    bytes of f32.
  - QK^T is computed per (query-superblock 512, key-block 128) into
    PSUM page tiles; exp of off-diag pages runs on ScalarE as ONE
    activation per page (multi-bank read) to amortize the fixed cost.
  - The 4 ragged diagonal blocks are packed flat and exp'd on the DVE
    by a Schraudolph fast-exp (see SCH_* below) whose bias tensor is
    also the causal mask -- one DVE op, no ScalarE, no mask multiplies.
  - P^T tiles feed PV matmuls as stationary weights; V carries an
    appended ones-column so the softmax denominator accumulates in the
    same PSUM tile (column 128). The 4 PV output slabs (129 cols each)
    are packed 2-per-bank into 2 PSUM banks via first-write start /
    last-write stop flags.
  - Items are phase-shifted: item n's PV matmuls are woven between item
    n+1's QK groups as in-order filler, so page-recycle and exp-latency
    waits never idle the PE (which would also drop its p-state clock).
  - out = PV / denom via one batched DVE reciprocal + one broadcast
    multiply per item.
"""

import numpy as np
import ml_dtypes

import concourse.bass as bass
import concourse.tile as tile
from concourse import bacc, mybir
from concourse.bass import ts
from concourse.bass_utils import run_bass_kernel_spmd
from concourse.masks import make_upper_triangular

B = 2
S = 2048
H = 32
HK = 8
D = 128
NCORES = 8
GH = H // NCORES  # query heads per core (= group size here)
SCALE = 0.08838834764831845  # 1/sqrt(128)

F32 = mybir.dt.float32
BF16 = mybir.dt.bfloat16
U16 = mybir.dt.uint16

NQB = S // 128  # 16 query/key blocks of 128
NSB = 4  # query superblocks of 512

# Schraudolph fast-exp on DVE: for bf16's bit layout,
# exp(s*SCALE) ~= bitcast_bf16(u16(round(s*SCH_A + SCH_B))) with
# SCH_A = SCALE*128*log2(e). C=7.4 calibrates the mantissa linear-interp
# sawtooth to zero mean multiplicative bias (rms err ~1.8%, which the
# shared softmax denominator partially cancels; end-to-end ~1.26e-2).
# For the diag pages the +B constant is a per-element TENSOR doubling as
# the causal mask: masked entries get B = -60000 so the affine result is
# negative and converts to (sat) 0 or (wrap) a 2^-100-scale denormal --
# either way an effective zero -- fusing exp+mask into one DVE op.
SCH_A = SCALE * 128.0 * 1.4426950408889634
SCH_B = 127.0 * 128.0 - 7.4
SCH_BMASK = -60000.0

# Scheduling knobs (tuned via TimelineSim sweep + HW validation).
CFG = dict(
    page=3,  # banks per off-diag QK page
    pbufs=2,  # page pool rotation depth (page*pbufs <= 6)
    hoist=True,
    diag_first=False,  # emit diag before the off-diag ki loop
    # off-diag pages drained on DVE via unmasked Schraudolph, as
    # (sq, gi) pairs; everything else drains on ScalarE. (3, 3) is the
    # last full page of sq3 items, where ACT is oversubscribed (the
    # measured exp(g3)/exp(g4)->QK-page-reuse stalls) and DVE has slack.
    dve_offdiag=((3, 3),),
    weave="uniform",  # 'uniform' | 'late' pending-PV placement
    po_bufs=1,  # PV accumulator buffers (2 requires page*pbufs <= 4)
    split_norm=False,  # recip+normalize per po bank (halves PV blocking)
    finish_before_diag=False,  # emit prev item's finish before the diag
    pv_order="bank_alternate",  # PV drain order; see sort below
    norm_div=False,  # fused divide: REJECTED by neuronxcc codegen
    norm_engine="dve",  # 'dve' | 'gpsimd' for the normalize multiply
    # 'dve': all diags Schraudolph; 'act': all diags ScalarE exp + DVE
    # mask muls; 'mixed': sq0 diags (the hoisted boundary ones) on DVE,
    # the rest on ScalarE -- trims ACT work exactly where it is the
    # bottleneck without adding mid-item cross-engine couplings.
    diag_engine="dve",
    early_finish=False,  # emit finish as soon as the last PV drains
    # drain full off-diag pages split: ACT takes the first 2 banks, DVE
    # (Schraudolph) concurrently takes bank 2 -> page frees ~430ns sooner
    split_page_drain=False,
    # sim-only probes (break numerics; never set for HW runs)
    probe_tiny_act=False,
    probe_tiny_dve=False,
    probe_no_norm=False,
)


def build_nc(cfg=None) -> bass.Bass:
    cfg = dict(CFG, **(cfg or {}))
    page = cfg["page"]
    pbufs = cfg["pbufs"]
    dve_off = set(map(tuple, cfg["dve_offdiag"]))
    assert page * pbufs + 2 * cfg["po_bufs"] <= 8

    nc = bacc.Bacc(
        "TRN2", target_bir_lowering=False, debug=False, num_devices=NCORES
    )
    # host-staged layouts: qT [b, g, d, s], kT [b, d, s], v [b, s, d]
    q_d = nc.declare_dram_parameter("query", [B, GH, D, S], BF16, isOutput=False)
    k_d = nc.declare_dram_parameter("key", [B, D, S], BF16, isOutput=False)
    v_d = nc.declare_dram_parameter("value", [B, S, D], BF16, isOutput=False)
    bm_d = nc.declare_dram_parameter("bmask", [128, 1280], F32, isOutput=False)
    o_d = nc.declare_dram_parameter("out", [B, S, GH, D], F32, isOutput=True)

    with tile.TileContext(nc) as tc:
        with (
            tc.tile_pool(name="consts", bufs=1) as consts,
            tc.tile_pool(name="pt", bufs=10) as pt_pool,
            tc.tile_pool(name="ptd", bufs=5) as ptd_pool,
            tc.tile_pool(name="osb", bufs=3) as osb_pool,
            tc.tile_pool(name="psum", bufs=1, space="PSUM") as psum_pool,
        ):
            # Schraudolph bias+mask tensor in the packed diag layout.
            bmask = consts.tile([128, 1280], F32)
            if cfg["diag_engine"] in ("act", "mixed"):
                # mask[k, q] = 1 where q >= k (keep), 0 above.
                mask = consts.tile([128, 128], BF16)
                make_upper_triangular(nc, mask, val=1.0, diag=True)

            kt_all = consts.tile([128, B, S], BF16)  # [d, b, k]
            qt_all = consts.tile([128, B * GH, S], BF16)  # [d, inst, q]
            v_ext = consts.tile([128, B, NQB, 132], BF16)  # [k, b, kblk, d+1]

            # PSUM: rotating QK page tiles + 2-bank packed PV acc.
            # Each page is its own pool tile so dependency tracking works
            # at page granularity: QK of a new group must not serialize
            # behind the exp of unrelated pages.
            def next_po():
                return psum_pool.tile(
                    [128, 2, 512], F32, tag="po", bufs=cfg["po_bufs"], name="po"
                )

            def next_page(banks=None):
                return psum_pool.tile(
                    [128, page, 512], F32, tag="page", bufs=pbufs, name="pg"
                )

            # ---- startup loads, first-needed first, in 512-col chunks so
            # item (0, sq) unblocks as soon as its kt/qt slices land.
            # bmask goes right after the first kt/qt pair: the first diag
            # QK only needs those, and bmask only gates the (DVE) exp.
            nc.vector.memset(v_ext[:, :, :, 128:129], 1.0)
            # first diag QK block needs only kt[0:128] + qt[0:512]; load
            # that first so the PE starts ~1.5us sooner.
            nc.sync.dma_start(out=kt_all[:, 0, 0:128], in_=k_d[0, :, 0:128])
            nc.sync.dma_start(out=qt_all[:, 0, 0:512], in_=q_d[0, 0, :, 0:512])
            nc.sync.dma_start(out=kt_all[:, 0, 128:512], in_=k_d[0, :, 128:512])
            nc.sync.dma_start(out=bmask[:, :], in_=bm_d[:, :])
            for c0 in range(512, 1536, 512):
                nc.sync.dma_start(
                    out=kt_all[:, 0, c0 : c0 + 512], in_=k_d[0, :, c0 : c0 + 512]
                )
                nc.sync.dma_start(
                    out=qt_all[:, 0, c0 : c0 + 512], in_=q_d[0, 0, :, c0 : c0 + 512]
                )
            nc.sync.dma_start(
                out=v_ext[:, 0, 0:8, 0:128],
                in_=v_d[0, 0:1024, :].rearrange("(n p) d -> p n d", p=128),
            )
            nc.sync.dma_start(
                out=kt_all[:, 0, 1536:2048], in_=k_d[0, :, 1536:2048]
            )
            nc.sync.dma_start(
                out=qt_all[:, 0, 1536:2048], in_=q_d[0, 0, :, 1536:2048]
            )
            nc.sync.dma_start(
                out=v_ext[:, 0, 8:16, 0:128],
                in_=v_d[0, 1024:2048, :].rearrange("(n p) d -> p n d", p=128),
            )
            nc.sync.dma_start(out=qt_all[:, 1, :], in_=q_d[0, 1, :, :])
            nc.sync.dma_start(out=kt_all[:, 1, :], in_=k_d[1, :, :])
            nc.sync.dma_start(
                out=v_ext[:, 1, :, 0:128],
                in_=v_d[1, :, :].rearrange("(n p) d -> p n d", p=128),
            )

            # pending = (pv_closures, finish_closure, cursor) of prev item
            pending = [None]

            def emit_pending_chunk(frac_done):
                if pending[0] is None:
                    return
                if cfg["weave"] == "late":
                    frac_done = frac_done * frac_done
                elif cfg["weave"] == "early":
                    frac_done = frac_done**0.5
                pvs, fin, cursor = pending[0]
                tgt = int(len(pvs) * frac_done + 0.5)
                while cursor[0] < tgt:
                    pvs[cursor[0]][2]()
                    cursor[0] += 1
                if cfg["early_finish"] and cursor[0] == len(pvs):
                    fin()
                    pending[0] = None

            def finish_pending():
                if pending[0] is None:
                    return
                pvs, fin, cursor = pending[0]
                while cursor[0] < len(pvs):
                    pvs[cursor[0]][2]()
                    cursor[0] += 1
                fin()
                pending[0] = None

            hoisted = {}

            def emit_diag(inst, sq):
                """QK + fused exp/mask (DVE Schraudolph) for the 4 ragged
                diagonal blocks, packed flat. Col mapping: ki+0 -> 0:512,
                ki+1 -> 512:896, ki+3 -> 896:1024, ki+2 -> 1024:1280.
                With 3-bank pages all 1280 cols live in one page (one DVE
                op); with 2-bank pages they split 1024 + 256 across two
                pages (two DVE ops). Returns the [128, 1280] u16 tile."""
                b, _g = divmod(inst, GH)
                q0 = 512 * sq
                kd = 4 * sq
                pg1 = next_page()
                nc.tensor.matmul(
                    pg1[:, 0, :],
                    lhsT=kt_all[:, b, ts(kd, 128)],
                    rhs=qt_all[:, inst, q0 : q0 + 512],
                    start=True, stop=True,
                )
                nc.tensor.matmul(
                    pg1[:, 1, 0:384],
                    lhsT=kt_all[:, b, ts(kd + 1, 128)],
                    rhs=qt_all[:, inst, q0 + 128 : q0 + 512],
                    start=True, stop=False,
                )
                nc.tensor.matmul(
                    pg1[:, 1, 384:512],
                    lhsT=kt_all[:, b, ts(kd + 3, 128)],
                    rhs=qt_all[:, inst, q0 + 384 : q0 + 512],
                    start=False, stop=True,
                )
                use_act = cfg["diag_engine"] == "act" or (
                    cfg["diag_engine"] == "mixed" and sq != 0
                )
                if use_act:
                    assert page >= 3
                    nc.tensor.matmul(
                        pg1[:, 2, 0:256],
                        lhsT=kt_all[:, b, ts(kd + 2, 128)],
                        rhs=qt_all[:, inst, q0 + 256 : q0 + 512],
                        start=True, stop=True,
                    )
                    ptd = ptd_pool.tile([128, 1280], BF16)
                    p1 = pg1[:, :, :]
                    nc.scalar.activation(
                        ptd[:, :],
                        bass.AP(p1.tensor, p1.offset, [list(p1.ap[0]), [1, 1280]]),
                        mybir.ActivationFunctionType.Exp,
                        scale=SCALE,
                    )
                    for c0 in (0, 512, 896, 1024):
                        nc.vector.tensor_mul(
                            ptd[:, c0 : c0 + 128], ptd[:, c0 : c0 + 128], mask
                        )
                    return ptd

                ptd = ptd_pool.tile([128, 1280], U16)
                p1 = pg1[:, :, :]
                if cfg["probe_tiny_dve"]:
                    nc.tensor.matmul(
                        (pg1 if page >= 3 else next_page())[:, page - 1 if page >= 3 else 0, 0:256],
                        lhsT=kt_all[:, b, ts(kd + 2, 128)],
                        rhs=qt_all[:, inst, q0 + 256 : q0 + 512],
                        start=True, stop=True,
                    )
                    nc.vector.scalar_tensor_tensor(
                        ptd[:, 0:8],
                        bass.AP(p1.tensor, p1.offset, [list(p1.ap[0]), [1, 8]]),
                        SCH_A,
                        bmask[:, 0:8],
                        mybir.AluOpType.mult,
                        mybir.AluOpType.add,
                    )
                    return ptd
                if page >= 3:
                    nc.tensor.matmul(
                        pg1[:, 2, 0:256],
                        lhsT=kt_all[:, b, ts(kd + 2, 128)],
                        rhs=qt_all[:, inst, q0 + 256 : q0 + 512],
                        start=True, stop=True,
                    )
                    nc.vector.scalar_tensor_tensor(
                        ptd[:, :],
                        bass.AP(p1.tensor, p1.offset, [list(p1.ap[0]), [1, 1280]]),
                        SCH_A,
                        bmask[:, :],
                        mybir.AluOpType.mult,
                        mybir.AluOpType.add,
                    )
                else:
                    pg2 = next_page()
                    nc.tensor.matmul(
                        pg2[:, 0, 0:256],
                        lhsT=kt_all[:, b, ts(kd + 2, 128)],
                        rhs=qt_all[:, inst, q0 + 256 : q0 + 512],
                        start=True, stop=True,
                    )
                    nc.vector.scalar_tensor_tensor(
                        ptd[:, 0:1024],
                        bass.AP(p1.tensor, p1.offset, [list(p1.ap[0]), [1, 1024]]),
                        SCH_A,
                        bmask[:, 0:1024],
                        mybir.AluOpType.mult,
                        mybir.AluOpType.add,
                    )
                    nc.vector.scalar_tensor_tensor(
                        ptd[:, 1024:1280],
                        pg2[:, 0, 0:256],
                        SCH_A,
                        bmask[:, 1024:1280],
                        mybir.AluOpType.mult,
                        mybir.AluOpType.add,
                    )
                return ptd

            def phase_item(inst, sq):
                b, g = divmod(inst, GH)
                q0 = 512 * sq
                noff = 4 * sq
                kd = 4 * sq
                ngroups = (noff + page - 1) // page + 1
                po = next_po()

                totals = [noff + 1 + j for j in range(4)]
                bank_tot = [totals[0] + totals[1], totals[2] + totals[3]]
                bank_cnt = [0, 0]
                pvs = []

                def defer_pv(pt_ap, kk, j):
                    def run(pt_ap=pt_ap, kk=kk, j=j):
                        bk = j // 2
                        off = (j % 2) * 256
                        bank_cnt[bk] += 1
                        nc.tensor.matmul(
                            po[:, bk, off : off + 129],
                            lhsT=pt_ap,
                            rhs=v_ext[:, b, kk, 0:129],
                            start=(bank_cnt[bk] == 1),
                            stop=(bank_cnt[bk] == bank_tot[bk]),
                        )
                    pvs.append((j, len(pvs), run))

                def emit_offdiag():
                    gi = 0
                    ki = 0
                    while ki < noff:
                        n = min(page, noff - ki)
                        pg = next_page()
                        for t in range(n):
                            nc.tensor.matmul(
                                pg[:, t, :],
                                lhsT=kt_all[:, b, ts(ki + t, 128)],
                                rhs=qt_all[:, inst, q0 : q0 + 512],
                                start=True,
                                stop=True,
                            )
                        pt = pt_pool.tile([128, page, 512], BF16)
                        if cfg["split_page_drain"] and n == page and page >= 3:
                            # concurrent drain: ACT takes banks 0..n-2,
                            # DVE (Schraudolph) takes the last bank, so
                            # the page frees at max(act, dve) not sum.
                            nc.scalar.activation(
                                pt[:, 0 : n - 1, :],
                                pg[:, 0 : n - 1, :],
                                mybir.ActivationFunctionType.Exp,
                                scale=SCALE,
                            )
                            nc.vector.tensor_scalar(
                                pt[:, n - 1, :].bitcast(U16),
                                pg[:, n - 1, :],
                                SCH_A,
                                SCH_B,
                                mybir.AluOpType.mult,
                                mybir.AluOpType.add,
                            )
                        elif (sq, gi) in dve_off:
                            # drain this page on the DVE (unmasked
                            # Schraudolph); error impact is tiny since
                            # off-diag rows have many keys.
                            nc.vector.tensor_scalar(
                                pt[:, 0:n, :].bitcast(U16),
                                pg[:, 0:n, :],
                                SCH_A,
                                SCH_B,
                                mybir.AluOpType.mult,
                                mybir.AluOpType.add,
                            )
                        else:
                            nc.scalar.activation(
                                pt[:, 0:n, 0:8] if cfg["probe_tiny_act"] else pt[:, 0:n, :],
                                pg[:, 0:n, 0:8] if cfg["probe_tiny_act"] else pg[:, 0:n, :],
                                mybir.ActivationFunctionType.Exp,
                                scale=SCALE,
                            )
                        for t in range(n):
                            for j in range(4):
                                defer_pv(pt[:, t, ts(j, 128)], ki + t, j)
                        ki += n
                        gi += 1
                        emit_pending_chunk(gi / ngroups)

                if cfg["diag_first"]:
                    if (inst, sq) in hoisted:
                        ptd = hoisted.pop((inst, sq))
                    else:
                        ptd = emit_diag(inst, sq)
                    emit_offdiag()
                else:
                    emit_offdiag()
                    if cfg["finish_before_diag"]:
                        finish_pending()
                    if (inst, sq) in hoisted:
                        ptd = hoisted.pop((inst, sq))
                    else:
                        ptd = emit_diag(inst, sq)

                for j in range(4):
                    defer_pv(ptd[:, 128 * j : 128 * (j + 1)].bitcast(BF16), kd, j)
                for j in range(1, 4):
                    defer_pv(
                        ptd[:, 512 + 128 * (j - 1) : 512 + 128 * j].bitcast(BF16),
                        kd + 1, j,
                    )
                for j in range(2, 4):
                    defer_pv(
                        ptd[:, 1024 + 128 * (j - 2) : 1024 + 128 * (j - 1)].bitcast(BF16),
                        kd + 2, j,
                    )
                defer_pv(ptd[:, 896:1024].bitcast(BF16), kd + 3, 3)

                # --- hoist the NEXT instance's sq0 diag QK+exp here, so
                # the engines roll straight through the instance boundary
                if cfg["hoist"] and sq == NSB - 1 and inst + 1 < B * GH:
                    hoisted[(inst + 1, 0)] = emit_diag(inst + 1, 0)

                # --- finish previous item (its remaining PV + normalize)
                if not cfg["finish_before_diag"]:
                    finish_pending()

                def norm_banks(o_sb, bk0, nbk):
                    # normalize slabs 2*bk0 .. 2*(bk0+nbk) of po: slab j
                    # sits at po offset j*256, denominator at col 128 of
                    # each slab. One reciprocal + one broadcast multiply
                    # per call; the reciprocals live in o_sb col 128 (not
                    # DMA'd out).
                    nsl = 2 * nbk
                    base = po[:, bk0:, :]
                    pp = list(base.ap[0])
                    den_ap = bass.AP(
                        base.tensor, base.offset + 128, [pp, [256, nsl], [1, 1]]
                    )
                    pv_ap = bass.AP(
                        base.tensor, base.offset, [pp, [256, nsl], [1, 128]]
                    )
                    recip = o_sb[:, 2 * bk0 : 2 * bk0 + nsl, 128:129]
                    rb = recip[:, :, :]
                    rb_b = bass.AP(
                        rb.tensor, rb.offset, [list(rb.ap[0]), [132, nsl], [0, 128]]
                    )
                    ob = o_sb[:, 2 * bk0 : 2 * bk0 + nsl, :]
                    ob3 = bass.AP(
                        ob.tensor, ob.offset, [list(ob.ap[0]), [132, nsl], [1, 128]]
                    )
                    if cfg["probe_no_norm"]:
                        nc.vector.tensor_mul(
                            o_sb[:, 0:1, 0:8], o_sb[:, 0:1, 0:8], o_sb[:, 0:1, 0:8]
                        )
                    elif cfg["norm_div"]:
                        # out = pv / den in one DVE op; den broadcast along
                        # the free dim straight from PSUM col 128.
                        den_b = bass.AP(
                            base.tensor,
                            base.offset + 128,
                            [pp, [256, nsl], [0, 128]],
                        )
                        nc.vector.tensor_tensor(
                            ob3, pv_ap, den_b, mybir.AluOpType.divide
                        )
                    else:
                        nc.vector.reciprocal(recip, den_ap)
                        if cfg["norm_engine"] == "gpsimd":
                            nc.gpsimd.tensor_mul(ob3, pv_ap, rb_b)
                        else:
                            nc.vector.tensor_mul(ob3, pv_ap, rb_b)

                def finish():
                    o_sb = osb_pool.tile([128, 4, 132], F32)
                    if cfg["split_norm"]:
                        norm_banks(o_sb, 0, 1)
                        norm_banks(o_sb, 1, 1)
                    else:
                        norm_banks(o_sb, 0, 2)
                    nc.sync.dma_start(
                        out=o_d[b, q0 : q0 + 512, g, :].rearrange(
                            "(n p) d -> p n d", p=128
                        ),
                        in_=o_sb[:, :, 0:128],
                    )

                if cfg["pv_order"] == "bank_major":
                    pvs.sort(key=lambda t: (t[0] // 2, t[1]))
                elif cfg["pv_order"] == "bank_alternate":
                    # j emission order 0,2,1,3 per k-block: consecutive PV
                    # matmuls then alternate PSUM banks (b0,b1,b0,b1)
                    # instead of b0,b0,b1,b1, avoiding back-to-back
                    # accumulating writes into the same bank.
                    order = {0: 0, 2: 1, 1: 2, 3: 3}
                    pvs.sort(key=lambda t: (t[1] // 4, order[t[0]]))
                pending[0] = (pvs, finish, [0])

            # qt for inst is loaded two instances ahead so QK never waits.
            # The last instance runs its superblocks big-to-small so the
            # pipeline tail drains the 10-matmul sq0 item, not sq3's 58.
            for inst in range(B * GH):
                if inst + 2 < B * GH:
                    bn, gn = divmod(inst + 2, GH)
                    nc.sync.dma_start(
                        out=qt_all[:, inst + 2, :], in_=q_d[bn, gn, :, :]
                    )
                for sq in range(NSB):
                    phase_item(inst, sq)
            finish_pending()

    nc.finalize()
    return nc


def _to_bf16(x):
    return np.asarray(x, dtype=np.float32).astype(ml_dtypes.bfloat16)


def _make_bmask():
    """Schraudolph bias + causal mask in the packed diag layout:
    block kd+0 at cols 0:512, kd+1 at 512:896, kd+3 at 896:1024,
    kd+2 at 1024:1280. In every range the keep condition reduces to
    (col - lo) >= partition, independent of sq."""
    bm = np.full((128, 1280), SCH_BMASK, np.float32)
    for lo, hi in ((0, 512), (512, 896), (896, 1024), (1024, 1280)):
        for p in range(128):
            bm[p, lo + p : hi] = SCH_B
    return bm


def make_in_maps(query, key, value):
    # host-side staging: bf16 cast + [d, s] transposes for Q and K
    qb = _to_bf16(query)  # [B, S, H, D]
    kb = _to_bf16(key)  # [B, S, HK, D]
    vb = _to_bf16(value)
    bm = _make_bmask()
    in_maps = []
    for c in range(NCORES):
        qt = np.ascontiguousarray(
            qb[:, :, GH * c : GH * (c + 1), :].transpose(0, 2, 3, 1)
        )  # [B, GH, D, S]
        kt = np.ascontiguousarray(kb[:, :, c, :].transpose(0, 2, 1))  # [B, D, S]
        vv = np.ascontiguousarray(vb[:, :, c, :])  # [B, S, D]
        in_maps.append({"query": qt, "key": kt, "value": vv, "bmask": bm})
    return in_maps


def kernel(query, key, value):
    nc = build_nc()
    res = run_bass_kernel_spmd(
        nc, make_in_maps(query, key, value), core_ids=list(range(NCORES))
    )
    outs = [np.asarray(res.results[c]["out"]) for c in range(NCORES)]
    return np.concatenate(outs, axis=2).astype(np.float32)


if __name__ == "__main__":
    rng = np.random.default_rng(0)
    q = rng.standard_normal((B, S, H, D), dtype=np.float32)
    k = rng.standard_normal((B, S, HK, D), dtype=np.float32)
    v = rng.standard_normal((B, S, HK, D), dtype=np.float32)
    out = kernel(q, k, v)
    print("out", out.shape, out.dtype, float(np.abs(out).max()))


# revision 58
# speedup vs baseline: 1.1863x; 1.1863x over previous
"""Causal GQA attention (prefill) on 8 TRN2 NeuronCores.

Problem: B=2, S=2048, H=32 query heads, Hk=8 kv heads, D=128, f32 I/O.
Sharding: tensor-parallel over heads -- core c gets query heads [4c, 4c+4)
and kv head c. Attention is fully independent per head: no collectives.

Per-core kernel: 8 instances of causal attention, one per (batch, qhead),
processed as a software-pipelined stream of 32 (instance, superblock)
items. Engine budget drives the design (PE matmul streaming ~117us is
the floor; ScalarE exp and the DVE share the elementwise work; the PE
is strictly in-order so the emission order IS the schedule):
  - Q and K are pre-cast to bf16 AND pre-transposed to [d, s] layout on
    the host, so Q^T/K^T stream straight from DRAM into SBUF with plain
    contiguous DMAs: zero PE transposes, zero DVE copies, half the DMA
    bytes of f32.
  - QK^T is computed per (query-superblock 512, key-block 128) into
    PSUM page tiles; exp of off-diag pages runs on ScalarE as ONE
    activation per page (multi-bank read) to amortize the fixed cost.
  - The 4 ragged diagonal blocks are packed flat and exp'd on the DVE
    by a Schraudolph fast-exp (see SCH_* below) whose bias tensor is
    also the causal mask -- one DVE op, no ScalarE, no mask multiplies.
  - P^T tiles feed PV matmuls as stationary weights; V carries an
    appended ones-column so the softmax denominator accumulates in the
    same PSUM tile (column 128). The 4 PV output slabs (129 cols each)
    are packed 2-per-bank into 2 PSUM banks via first-write start /
    last-write stop flags.
  - Items are phase-shifted: item n's PV matmuls are woven between item
    n+1's QK groups as in-order filler, so page-recycle and exp-latency
    waits never idle the PE (which would also drop its p-state clock).
  - out = PV / denom via one batched DVE reciprocal + one broadcast
    multiply per item.
"""

import numpy as np
import ml_dtypes

import concourse.bass as bass
import concourse.tile as tile
from concourse import bacc, mybir
from concourse.bass import ts
from concourse.bass_utils import run_bass_kernel_spmd
from concourse.masks import make_upper_triangular

B = 2
S = 2048
H = 32
HK = 8
D = 128
NCORES = 8
GH = H // NCORES  # query heads per core (= group size here)
SCALE = 0.08838834764831845  # 1/sqrt(128)

F32 = mybir.dt.float32
BF16 = mybir.dt.bfloat16
U16 = mybir.dt.uint16

NQB = S // 128  # 16 query/key blocks of 128
NSB = 4  # query superblocks of 512

# Schraudolph fast-exp on DVE: for bf16's bit layout,
# exp(s*SCALE) ~= bitcast_bf16(u16(round(s*SCH_A + SCH_B))) with
# SCH_A = SCALE*128*log2(e). C=7.4 calibrates the mantissa linear-interp
# sawtooth to zero mean multiplicative bias (rms err ~1.8%, which the
# shared softmax denominator partially cancels; end-to-end ~1.26e-2).
# For the diag pages the +B constant is a per-element TENSOR doubling as
# the causal mask: masked entries get B = -60000 so the affine result is
# negative and converts to (sat) 0 or (wrap) a 2^-100-scale denormal --
# either way an effective zero -- fusing exp+mask into one DVE op.
SCH_A = SCALE * 128.0 * 1.4426950408889634
SCH_B = 127.0 * 128.0 - 7.4
SCH_BMASK = -60000.0

# Scheduling knobs (tuned via TimelineSim sweep + HW validation).
CFG = dict(
    page=3,  # banks per off-diag QK page
    pbufs=2,  # page pool rotation depth (page*pbufs <= 6)
    hoist=True,
    diag_first=False,  # emit diag before the off-diag ki loop
    # off-diag pages drained on DVE via unmasked Schraudolph, as
    # (sq, gi) pairs; everything else drains on ScalarE.
    dve_offdiag=(),
    weave="uniform",  # 'uniform' | 'late' pending-PV placement
    po_bufs=1,  # PV accumulator buffers (2 requires page*pbufs <= 4)
    split_norm=False,  # recip+normalize per po bank (halves PV blocking)
    finish_before_diag=False,  # emit prev item's finish before the diag
    pv_order="k_major",  # PV drain order; bank_alternate measured neutral
    norm_div=False,  # fused divide: REJECTED by neuronxcc codegen
    norm_engine="dve",  # 'dve' | 'gpsimd' for the normalize multiply
    # 'dve': all diags Schraudolph; 'act': all diags ScalarE exp + DVE
    # mask muls; 'mixed': sq0 diags (the hoisted boundary ones) on DVE,
    # the rest on ScalarE -- trims ACT work exactly where it is the
    # bottleneck without adding mid-item cross-engine couplings.
    diag_engine="mixed",
    early_finish=False,  # emit finish as soon as the last PV drains
    # drain full off-diag pages split: ACT takes the first 2 banks, DVE
    # (Schraudolph) concurrently takes bank 2 -> page frees ~430ns sooner
    split_page_drain=False,
    # sim-only probes (break numerics; never set for HW runs)
    probe_tiny_act=False,
    probe_tiny_dve=False,
    probe_no_norm=False,
)


def build_nc(cfg=None) -> bass.Bass:
    cfg = dict(CFG, **(cfg or {}))
    page = cfg["page"]
    pbufs = cfg["pbufs"]
    dve_off = set(map(tuple, cfg["dve_offdiag"]))
    assert page * pbufs + 2 * cfg["po_bufs"] <= 8

    nc = bacc.Bacc(
        "TRN2", target_bir_lowering=False, debug=False, num_devices=NCORES
    )
    # host-staged layouts: qT [b, g, d, s], kT [b, d, s], v [b, s, d]
    q_d = nc.declare_dram_parameter("query", [B, GH, D, S], BF16, isOutput=False)
    k_d = nc.declare_dram_parameter("key", [B, D, S], BF16, isOutput=False)
    v_d = nc.declare_dram_parameter("value", [B, S, D], BF16, isOutput=False)
    bm_d = nc.declare_dram_parameter("bmask", [128, 1280], F32, isOutput=False)
    o_d = nc.declare_dram_parameter("out", [B, S, GH, D], F32, isOutput=True)

    with tile.TileContext(nc) as tc:
        with (
            tc.tile_pool(name="consts", bufs=1) as consts,
            tc.tile_pool(name="pt", bufs=10) as pt_pool,
            tc.tile_pool(name="ptd", bufs=5) as ptd_pool,
            tc.tile_pool(name="osb", bufs=3) as osb_pool,
            tc.tile_pool(name="psum", bufs=1, space="PSUM") as psum_pool,
        ):
            # Schraudolph bias+mask tensor in the packed diag layout.
            bmask = consts.tile([128, 1280], F32)
            if cfg["diag_engine"] in ("act", "mixed"):
                # mask[k, q] = 1 where q >= k (keep), 0 above.
                mask = consts.tile([128, 128], BF16)
                make_upper_triangular(nc, mask, val=1.0, diag=True)

            kt_all = consts.tile([128, B, S], BF16)  # [d, b, k]
            qt_all = consts.tile([128, B * GH, S], BF16)  # [d, inst, q]
            v_ext = consts.tile([128, B, NQB, 132], BF16)  # [k, b, kblk, d+1]

            # PSUM: rotating QK page tiles + 2-bank packed PV acc.
            # Each page is its own pool tile so dependency tracking works
            # at page granularity: QK of a new group must not serialize
            # behind the exp of unrelated pages.
            def next_po():
                return psum_pool.tile(
                    [128, 2, 512], F32, tag="po", bufs=cfg["po_bufs"], name="po"
                )

            def next_page(banks=None):
                return psum_pool.tile(
                    [128, page, 512], F32, tag="page", bufs=pbufs, name="pg"
                )

            # ---- startup loads, first-needed first, in 512-col chunks so
            # item (0, sq) unblocks as soon as its kt/qt slices land.
            # bmask goes right after the first kt/qt pair: the first diag
            # QK only needs those, and bmask only gates the (DVE) exp.
            nc.vector.memset(v_ext[:, :, :, 128:129], 1.0)
            # first diag QK block needs only kt[0:128] + qt[0:512]; load
            # that first so the PE starts ~1.5us sooner.
            nc.sync.dma_start(out=kt_all[:, 0, 0:128], in_=k_d[0, :, 0:128])
            nc.sync.dma_start(out=qt_all[:, 0, 0:512], in_=q_d[0, 0, :, 0:512])
            nc.sync.dma_start(out=kt_all[:, 0, 128:512], in_=k_d[0, :, 128:512])
            nc.sync.dma_start(out=bmask[:, :], in_=bm_d[:, :])
            for c0 in range(512, 1536, 512):
                nc.sync.dma_start(
                    out=kt_all[:, 0, c0 : c0 + 512], in_=k_d[0, :, c0 : c0 + 512]
                )
                nc.sync.dma_start(
                    out=qt_all[:, 0, c0 : c0 + 512], in_=q_d[0, 0, :, c0 : c0 + 512]
                )
            nc.sync.dma_start(
                out=v_ext[:, 0, 0:8, 0:128],
                in_=v_d[0, 0:1024, :].rearrange("(n p) d -> p n d", p=128),
            )
            nc.sync.dma_start(
                out=kt_all[:, 0, 1536:2048], in_=k_d[0, :, 1536:2048]
            )
            nc.sync.dma_start(
                out=qt_all[:, 0, 1536:2048], in_=q_d[0, 0, :, 1536:2048]
            )
            nc.sync.dma_start(
                out=v_ext[:, 0, 8:16, 0:128],
                in_=v_d[0, 1024:2048, :].rearrange("(n p) d -> p n d", p=128),
            )
            nc.sync.dma_start(out=qt_all[:, 1, :], in_=q_d[0, 1, :, :])
            nc.sync.dma_start(out=kt_all[:, 1, :], in_=k_d[1, :, :])
            nc.sync.dma_start(
                out=v_ext[:, 1, :, 0:128],
                in_=v_d[1, :, :].rearrange("(n p) d -> p n d", p=128),
            )

            # pending = (pv_closures, finish_closure, cursor) of prev item
            pending = [None]

            def emit_pending_chunk(frac_done):
                if pending[0] is None:
                    return
                if cfg["weave"] == "late":
                    frac_done = frac_done * frac_done
                elif cfg["weave"] == "early":
                    frac_done = frac_done**0.5
                pvs, fin, cursor = pending[0]
                tgt = int(len(pvs) * frac_done + 0.5)
                while cursor[0] < tgt:
                    pvs[cursor[0]][2]()
                    cursor[0] += 1
                if cfg["early_finish"] and cursor[0] == len(pvs):
                    fin()
                    pending[0] = None

            def finish_pending():
                if pending[0] is None:
                    return
                pvs, fin, cursor = pending[0]
                while cursor[0] < len(pvs):
                    pvs[cursor[0]][2]()
                    cursor[0] += 1
                fin()
                pending[0] = None

            hoisted = {}

            def emit_diag(inst, sq):
                """QK + fused exp/mask (DVE Schraudolph) for the 4 ragged
                diagonal blocks, packed flat. Col mapping: ki+0 -> 0:512,
                ki+1 -> 512:896, ki+3 -> 896:1024, ki+2 -> 1024:1280.
                With 3-bank pages all 1280 cols live in one page (one DVE
                op); with 2-bank pages they split 1024 + 256 across two
                pages (two DVE ops). Returns the [128, 1280] u16 tile."""
                b, _g = divmod(inst, GH)
                q0 = 512 * sq
                kd = 4 * sq
                pg1 = next_page()
                nc.tensor.matmul(
                    pg1[:, 0, :],
                    lhsT=kt_all[:, b, ts(kd, 128)],
                    rhs=qt_all[:, inst, q0 : q0 + 512],
                    start=True, stop=True,
                )
                nc.tensor.matmul(
                    pg1[:, 1, 0:384],
                    lhsT=kt_all[:, b, ts(kd + 1, 128)],
                    rhs=qt_all[:, inst, q0 + 128 : q0 + 512],
                    start=True, stop=False,
                )
                nc.tensor.matmul(
                    pg1[:, 1, 384:512],
                    lhsT=kt_all[:, b, ts(kd + 3, 128)],
                    rhs=qt_all[:, inst, q0 + 384 : q0 + 512],
                    start=False, stop=True,
                )
                use_act = cfg["diag_engine"] == "act" or (
                    cfg["diag_engine"] == "mixed" and sq != 0
                )
                if use_act:
                    assert page >= 3
                    nc.tensor.matmul(
                        pg1[:, 2, 0:256],
                        lhsT=kt_all[:, b, ts(kd + 2, 128)],
                        rhs=qt_all[:, inst, q0 + 256 : q0 + 512],
                        start=True, stop=True,
                    )
                    ptd = ptd_pool.tile([128, 1280], BF16)
                    p1 = pg1[:, :, :]
                    nc.scalar.activation(
                        ptd[:, :],
                        bass.AP(p1.tensor, p1.offset, [list(p1.ap[0]), [1, 1280]]),
                        mybir.ActivationFunctionType.Exp,
                        scale=SCALE,
                    )
                    for c0 in (0, 512, 896, 1024):
                        nc.vector.tensor_mul(
                            ptd[:, c0 : c0 + 128], ptd[:, c0 : c0 + 128], mask
                        )
                    return ptd

                ptd = ptd_pool.tile([128, 1280], U16)
                p1 = pg1[:, :, :]
                if cfg["probe_tiny_dve"]:
                    nc.tensor.matmul(
                        (pg1 if page >= 3 else next_page())[:, page - 1 if page >= 3 else 0, 0:256],
                        lhsT=kt_all[:, b, ts(kd + 2, 128)],
                        rhs=qt_all[:, inst, q0 + 256 : q0 + 512],
                        start=True, stop=True,
                    )
                    nc.vector.scalar_tensor_tensor(
                        ptd[:, 0:8],
                        bass.AP(p1.tensor, p1.offset, [list(p1.ap[0]), [1, 8]]),
                        SCH_A,
                        bmask[:, 0:8],
                        mybir.AluOpType.mult,
                        mybir.AluOpType.add,
                    )
                    return ptd
                if page >= 3:
                    nc.tensor.matmul(
                        pg1[:, 2, 0:256],
                        lhsT=kt_all[:, b, ts(kd + 2, 128)],
                        rhs=qt_all[:, inst, q0 + 256 : q0 + 512],
                        start=True, stop=True,
                    )
                    nc.vector.scalar_tensor_tensor(
                        ptd[:, :],
                        bass.AP(p1.tensor, p1.offset, [list(p1.ap[0]), [1, 1280]]),
                        SCH_A,
                        bmask[:, :],
                        mybir.AluOpType.mult,
                        mybir.AluOpType.add,
                    )
                else:
                    pg2 = next_page()
                    nc.tensor.matmul(
                        pg2[:, 0, 0:256],
                        lhsT=kt_all[:, b, ts(kd + 2, 128)],
                        rhs=qt_all[:, inst, q0 + 256 : q0 + 512],
                        start=True, stop=True,
                    )
                    nc.vector.scalar_tensor_tensor(
                        ptd[:, 0:1024],
                        bass.AP(p1.tensor, p1.offset, [list(p1.ap[0]), [1, 1024]]),
                        SCH_A,
                        bmask[:, 0:1024],
                        mybir.AluOpType.mult,
                        mybir.AluOpType.add,
                    )
                    nc.vector.scalar_tensor_tensor(
                        ptd[:, 1024:1280],
                        pg2[:, 0, 0:256],
                        SCH_A,
                        bmask[:, 1024:1280],
                        mybir.AluOpType.mult,
                        mybir.AluOpType.add,
                    )
                return ptd

            def phase_item(inst, sq):
                b, g = divmod(inst, GH)
                q0 = 512 * sq
                noff = 4 * sq
                kd = 4 * sq
                ngroups = (noff + page - 1) // page + 1
                po = next_po()

                totals = [noff + 1 + j for j in range(4)]
                bank_tot = [totals[0] + totals[1], totals[2] + totals[3]]
                bank_cnt = [0, 0]
                pvs = []

                def defer_pv(pt_ap, kk, j):
                    def run(pt_ap=pt_ap, kk=kk, j=j):
                        bk = j // 2
                        off = (j % 2) * 256
                        bank_cnt[bk] += 1
                        nc.tensor.matmul(
                            po[:, bk, off : off + 129],
                            lhsT=pt_ap,
                            rhs=v_ext[:, b, kk, 0:129],
                            start=(bank_cnt[bk] == 1),
                            stop=(bank_cnt[bk] == bank_tot[bk]),
                        )
                    pvs.append((j, len(pvs), run))

                def emit_offdiag():
                    gi = 0
                    ki = 0
                    while ki < noff:
                        n = min(page, noff - ki)
                        pg = next_page()
                        for t in range(n):
                            nc.tensor.matmul(
                                pg[:, t, :],
                                lhsT=kt_all[:, b, ts(ki + t, 128)],
                                rhs=qt_all[:, inst, q0 : q0 + 512],
                                start=True,
                                stop=True,
                            )
                        pt = pt_pool.tile([128, page, 512], BF16)
                        if cfg["split_page_drain"] and n == page and page >= 3:
                            # concurrent drain: ACT takes banks 0..n-2,
                            # DVE (Schraudolph) takes the last bank, so
                            # the page frees at max(act, dve) not sum.
                            nc.scalar.activation(
                                pt[:, 0 : n - 1, :],
                                pg[:, 0 : n - 1, :],
                                mybir.ActivationFunctionType.Exp,
                                scale=SCALE,
                            )
                            nc.vector.tensor_scalar(
                                pt[:, n - 1, :].bitcast(U16),
                                pg[:, n - 1, :],
                                SCH_A,
                                SCH_B,
                                mybir.AluOpType.mult,
                                mybir.AluOpType.add,
                            )
                        elif (sq, gi) in dve_off:
                            # drain this page on the DVE (unmasked
                            # Schraudolph); error impact is tiny since
                            # off-diag rows have many keys.
                            nc.vector.tensor_scalar(
                                pt[:, 0:n, :].bitcast(U16),
                                pg[:, 0:n, :],
                                SCH_A,
                                SCH_B,
                                mybir.AluOpType.mult,
                                mybir.AluOpType.add,
                            )
                        else:
                            nc.scalar.activation(
                                pt[:, 0:n, 0:8] if cfg["probe_tiny_act"] else pt[:, 0:n, :],
                                pg[:, 0:n, 0:8] if cfg["probe_tiny_act"] else pg[:, 0:n, :],
                                mybir.ActivationFunctionType.Exp,
                                scale=SCALE,
                            )
                        for t in range(n):
                            for j in range(4):
                                defer_pv(pt[:, t, ts(j, 128)], ki + t, j)
                        ki += n
                        gi += 1
                        emit_pending_chunk(gi / ngroups)

                if cfg["diag_first"]:
                    if (inst, sq) in hoisted:
                        ptd = hoisted.pop((inst, sq))
                    else:
                        ptd = emit_diag(inst, sq)
                    emit_offdiag()
                else:
                    emit_offdiag()
                    if cfg["finish_before_diag"]:
                        finish_pending()
                    if (inst, sq) in hoisted:
                        ptd = hoisted.pop((inst, sq))
                    else:
                        ptd = emit_diag(inst, sq)

                for j in range(4):
                    defer_pv(ptd[:, 128 * j : 128 * (j + 1)].bitcast(BF16), kd, j)
                for j in range(1, 4):
                    defer_pv(
                        ptd[:, 512 + 128 * (j - 1) : 512 + 128 * j].bitcast(BF16),
                        kd + 1, j,
                    )
                for j in range(2, 4):
                    defer_pv(
                        ptd[:, 1024 + 128 * (j - 2) : 1024 + 128 * (j - 1)].bitcast(BF16),
                        kd + 2, j,
                    )
                defer_pv(ptd[:, 896:1024].bitcast(BF16), kd + 3, 3)

                # --- hoist the NEXT instance's sq0 diag QK+exp here, so
                # the engines roll straight through the instance boundary
                if cfg["hoist"] and sq == NSB - 1 and inst + 1 < B * GH:
                    hoisted[(inst + 1, 0)] = emit_diag(inst + 1, 0)

                # --- finish previous item (its remaining PV + normalize)
                if not cfg["finish_before_diag"]:
                    finish_pending()

                def norm_banks(o_sb, bk0, nbk):
                    # normalize slabs 2*bk0 .. 2*(bk0+nbk) of po: slab j
                    # sits at po offset j*256, denominator at col 128 of
                    # each slab. One reciprocal + one broadcast multiply
                    # per call; the reciprocals live in o_sb col 128 (not
                    # DMA'd out).
                    nsl = 2 * nbk
                    base = po[:, bk0:, :]
                    pp = list(base.ap[0])
                    den_ap = bass.AP(
                        base.tensor, base.offset + 128, [pp, [256, nsl], [1, 1]]
                    )
                    pv_ap = bass.AP(
                        base.tensor, base.offset, [pp, [256, nsl], [1, 128]]
                    )
                    recip = o_sb[:, 2 * bk0 : 2 * bk0 + nsl, 128:129]
                    rb = recip[:, :, :]
                    rb_b = bass.AP(
                        rb.tensor, rb.offset, [list(rb.ap[0]), [132, nsl], [0, 128]]
                    )
                    ob = o_sb[:, 2 * bk0 : 2 * bk0 + nsl, :]
                    ob3 = bass.AP(
                        ob.tensor, ob.offset, [list(ob.ap[0]), [132, nsl], [1, 128]]
                    )
                    if cfg["probe_no_norm"]:
                        nc.vector.tensor_mul(
                            o_sb[:, 0:1, 0:8], o_sb[:, 0:1, 0:8], o_sb[:, 0:1, 0:8]
                        )
                    elif cfg["norm_div"]:
                        # out = pv / den in one DVE op; den broadcast along
                        # the free dim straight from PSUM col 128.
                        den_b = bass.AP(
                            base.tensor,
                            base.offset + 128,
                            [pp, [256, nsl], [0, 128]],
                        )
                        nc.vector.tensor_tensor(
                            ob3, pv_ap, den_b, mybir.AluOpType.divide
                        )
                    else:
                        nc.vector.reciprocal(recip, den_ap)
                        if cfg["norm_engine"] == "gpsimd":
                            nc.gpsimd.tensor_mul(ob3, pv_ap, rb_b)
                        else:
                            nc.vector.tensor_mul(ob3, pv_ap, rb_b)

                def finish():
                    o_sb = osb_pool.tile([128, 4, 132], F32)
                    if cfg["split_norm"]:
                        norm_banks(o_sb, 0, 1)
                        norm_banks(o_sb, 1, 1)
                    else:
                        norm_banks(o_sb, 0, 2)
                    nc.sync.dma_start(
                        out=o_d[b, q0 : q0 + 512, g, :].rearrange(
                            "(n p) d -> p n d", p=128
                        ),
                        in_=o_sb[:, :, 0:128],
                    )

                if cfg["pv_order"] == "bank_major":
                    pvs.sort(key=lambda t: (t[0] // 2, t[1]))
                elif cfg["pv_order"] == "bank_alternate":
                    # j emission order 0,2,1,3 per k-block: consecutive PV
                    # matmuls then alternate PSUM banks (b0,b1,b0,b1)
                    # instead of b0,b0,b1,b1, avoiding back-to-back
                    # accumulating writes into the same bank.
                    order = {0: 0, 2: 1, 1: 2, 3: 3}
                    pvs.sort(key=lambda t: (t[1] // 4, order[t[0]]))
                pending[0] = (pvs, finish, [0])

            # qt for inst is loaded two instances ahead so QK never waits.
            # The last instance runs its superblocks big-to-small so the
            # pipeline tail drains the 10-matmul sq0 item, not sq3's 58.
            for inst in range(B * GH):
                if inst + 2 < B * GH:
                    bn, gn = divmod(inst + 2, GH)
                    nc.sync.dma_start(
                        out=qt_all[:, inst + 2, :], in_=q_d[bn, gn, :, :]
                    )
                for sq in range(NSB):
                    phase_item(inst, sq)
            finish_pending()

    nc.finalize()
    return nc


def _to_bf16(x):
    return np.asarray(x, dtype=np.float32).astype(ml_dtypes.bfloat16)


def _make_bmask():
    """Schraudolph bias + causal mask in the packed diag layout:
    block kd+0 at cols 0:512, kd+1 at 512:896, kd+3 at 896:1024,
    kd+2 at 1024:1280. In every range the keep condition reduces to
    (col - lo) >= partition, independent of sq."""
    bm = np.full((128, 1280), SCH_BMASK, np.float32)
    for lo, hi in ((0, 512), (512, 896), (896, 1024), (1024, 1280)):
        for p in range(128):
            bm[p, lo + p : hi] = SCH_B
    return bm


def make_in_maps(query, key, value):
    # host-side staging: bf16 cast + [d, s] transposes for Q and K
    qb = _to_bf16(query)  # [B, S, H, D]
    kb = _to_bf16(key)  # [B, S, HK, D]
    vb = _to_bf16(value)
    bm = _make_bmask()
    in_maps = []
    for c in range(NCORES):
        qt = np.ascontiguousarray(
            qb[:, :, GH * c : GH * (c + 1), :].transpose(0, 2, 3, 1)
        )  # [B, GH, D, S]
        kt = np.ascontiguousarray(kb[:, :, c, :].transpose(0, 2, 1))  # [B, D, S]
        vv = np.ascontiguousarray(vb[:, :, c, :])  # [B, S, D]
        in_maps.append({"query": qt, "key": kt, "value": vv, "bmask": bm})
    return in_maps


def kernel(query, key, value):
    nc = build_nc()
    res = run_bass_kernel_spmd(
        nc, make_in_maps(query, key, value), core_ids=list(range(NCORES))
    )
    outs = [np.asarray(res.results[c]["out"]) for c in range(NCORES)]
    return np.concatenate(outs, axis=2).astype(np.float32)


if __name__ == "__main__":
    rng = np.random.default_rng(0)
    q = rng.standard_normal((B, S, H, D), dtype=np.float32)
    k = rng.standard_normal((B, S, HK, D), dtype=np.float32)
    v = rng.standard_normal((B, S, HK, D), dtype=np.float32)
    out = kernel(q, k, v)
    print("out", out.shape, out.dtype, float(np.abs(out).max()))
